# revision 7
# baseline (speedup 1.0000x reference)
"""MixerBlock Trainium2 kernel — 8-core data-parallel over batch.

Per core: one batch element (T=2048, E=1024), f32 in/out.
  1. LN1 (stats+apply, tokens on partitions)
  2. PE-transpose h -> hT (features on partitions)
  3. per-head projection p = h @ Wp  (heads concatenated, Wp host-folded)
  4. causal decay mixing: M = D_pre * C * D_post factorization ->
     shared causal-ones matmuls + running-carry cumsum across 512-blocks
  5. out-proj + residual
  6. LN2, PE-transpose, FF1 (+gelu fused in ACT eviction), FF2 + residual
All heavy matmuls run in float32r (1 cycle/row at N>=256).
Host folds: LN gains/biases into adjacent weights; decay powers into
pre/post diagonal scale vectors (exact for d=1, which clip(ones)=1 gives).
"""

import numpy as np

B, T, E = 8, 2048, 1024
H = 16
HD = E // H
DFF = 4 * E
DC = T // 512
EPS = 1e-5
NCORES = 8
P = 128
TT = T // P           # 16 token tiles
ET = E // P           # 8 feature tiles
MT = DFF // P         # 32 ff tiles
NPAIR = H // 2        # 8 head pairs (2 heads of 64 features = 128 partitions)
SB = 512              # s-block width (one psum bank of f32)
NSB = T // SB         # 4 s-blocks
TB = 4                # ff token-block = TB*128 = 512 tokens
NTB = TT // TB        # 4 ff token blocks

_CACHE = {}
GELU_AF = "Gelu_apprx_tanh"  # test.py sim mode overrides to "Copy"


def _build(flags):
    (need_pre_col, need_post_row, need_pbias, need_opbias, need_b2) = flags
    import concourse.bacc as bacc
    import concourse.tile as tile
    from concourse import mybir
    from contextlib import ExitStack

    F32 = mybir.dt.float32
    F32R = mybir.dt.float32r
    AF = mybir.ActivationFunctionType

    nc = bacc.Bacc("TRN2", target_bir_lowering=False)

    x_d = nc.dram_tensor("x", [T, E], F32, kind="ExternalInput")
    wp_d = nc.dram_tensor("wp", [E, E], F32R, kind="ExternalInput")
    ow_d = nc.dram_tensor("ow", [E, E], F32R, kind="ExternalInput")
    w1_d = nc.dram_tensor("w1", [E, DFF], F32R, kind="ExternalInput")
    w2_d = nc.dram_tensor("w2", [DFF, E], F32R, kind="ExternalInput")
    c_d = nc.dram_tensor("cfull", [P, SB], F32R, kind="ExternalInput")
    id_d = nc.dram_tensor("ident", [P, P], F32R, kind="ExternalInput")
    pre_d = nc.dram_tensor("pret", [T, H], F32, kind="ExternalInput")
    pc_d = nc.dram_tensor("postc", [H // 2, T], F32, kind="ExternalInput")
    b1_d = nc.dram_tensor("b1t", [P, MT], F32, kind="ExternalInput")
    if need_post_row:
        pr_d = nc.dram_tensor("postr", [H // 2, T], F32, kind="ExternalInput")
    if need_pbias:
        pb_d = nc.dram_tensor("pbias", [1, E], F32, kind="ExternalInput")
    if need_opbias:
        obl_d = nc.dram_tensor("oblhs", [32, T], F32R, kind="ExternalInput")
        obr_d = nc.dram_tensor("obrhs", [32, E], F32R, kind="ExternalInput")
    if need_b2:
        b2_d = nc.dram_tensor("b2", [1, E], F32, kind="ExternalInput")
    out_d = nc.dram_tensor("out", [T, E], F32, kind="ExternalOutput")

    with tile.TileContext(nc) as tc, ExitStack() as top:
        consts = top.enter_context(tc.tile_pool(name="consts", bufs=1))
        cfull = consts.tile([P, SB], F32R, tag="cfull")
        ident = consts.tile([P, P], F32R, tag="ident")
        pret = consts.tile([P, TT, H], F32, tag="pret")
        b1t = consts.tile([P, MT], F32, tag="b1t")
        epst = consts.tile([P, 1], F32, tag="eps")
        nc.sync.dma_start(out=cfull, in_=c_d[:])
        nc.sync.dma_start(out=ident, in_=id_d[:])
        nc.sync.dma_start(out=pret, in_=pre_d[:].rearrange("(tt p) h -> p tt h", p=P))
        nc.sync.dma_start(out=b1t, in_=b1_d[:])
        nc.vector.memset(epst, EPS)
        if need_pbias:
            pbias = consts.tile([P, E], F32, tag="pbias")
            nc.gpsimd.dma_start(out=pbias, in_=pb_d[0, :].partition_broadcast(P))
        if need_b2:
            b2b = consts.tile([P, E], F32, tag="b2b")
            nc.gpsimd.dma_start(out=b2b, in_=b2_d[0, :].partition_broadcast(P))
        if need_opbias:
            obl = consts.tile([32, T], F32R, tag="obl")
            obr = consts.tile([32, E], F32R, tag="obr")
            nc.sync.dma_start(out=obl, in_=obl_d[:])
            nc.sync.dma_start(out=obr, in_=obr_d[:])

        mainps = top.enter_context(tc.tile_pool(name="mainps", bufs=6, space="PSUM"))
        tps = top.enter_context(tc.tile_pool(name="tps", bufs=2, space="PSUM"))
        small = top.enter_context(tc.tile_pool(name="small", bufs=10))

        def layernorm(x_t, h_t, pool):
            """LN stats over free dim + apply; h_t = (x-mu)*rstd (gain/bias folded)."""
            stats = pool.tile([P, 2, 6], F32, tag="bnstats")
            mv = pool.tile([P, 2], F32, tag="bnmv")
            for g in range(2):
                nc.vector.bn_stats(out=stats[:, g, :], in_=x_t[:, g * 512:(g + 1) * 512])
            nc.vector.bn_aggr(out=mv, in_=stats)
            rstd = pool.tile([P, 1], F32, tag="rstd")
            nc.scalar.activation(out=rstd, in_=mv[:, 1:2], func=AF.Sqrt,
                                 bias=epst, scale=1.0)
            nc.vector.reciprocal(out=rstd, in_=rstd)
            nc.vector.tensor_scalar(out=h_t, in0=x_t, scalar1=mv[:, 0:1],
                                    scalar2=rstd, op0=mybir.AluOpType.subtract,
                                    op1=mybir.AluOpType.mult)

        # ---------------- phase 1: LN1 + transpose + projection ----------------
        s1 = ExitStack()   # proj-only pools: closed after phase 1
        sp = ExitStack()   # p_all: closed after phase 2
        sm = ExitStack()   # mixed (+ col scales): closed after phase 3
        ppool = sp.enter_context(tc.tile_pool(name="ppool", bufs=1))
        p_all = ppool.tile([P, TT, E], F32R, tag="p")
        with s1 as ph:
            wpool = ph.enter_context(tc.tile_pool(name="wpool", bufs=1))
            w_sb = wpool.tile([P, ET, E], F32R, tag="w")
            nc.sync.dma_start(out=w_sb, in_=wp_d[:].rearrange("(et p) f -> p et f", p=P))

            xin = ph.enter_context(tc.tile_pool(name="xin", bufs=3))
            hp = ph.enter_context(tc.tile_pool(name="hp", bufs=2))
            htp = ph.enter_context(tc.tile_pool(name="htp", bufs=2))

            for tt in range(TT):
                x_t = xin.tile([P, E], F32, tag="x")
                nc.sync.dma_start(out=x_t, in_=x_d[tt * P:(tt + 1) * P, :])
                h_t = hp.tile([P, E], F32R, tag="h")
                layernorm(x_t, h_t, small)
                ht_t = htp.tile([P, ET, P], F32R, tag="ht")
                for ec in range(ET):
                    pst = tps.tile([P, P], F32R, tag="tp")
                    nc.tensor.transpose(pst[:], h_t[:, ec * P:(ec + 1) * P], ident[:])
                    nc.scalar.copy(out=ht_t[:, ec, :], in_=pst[:])
                for jb in range(2):
                    ps = mainps.tile([P, SB], F32, tag="mm")
                    for et in range(ET):
                        nc.tensor.matmul(ps[:], ht_t[:, et, :],
                                         w_sb[:, et, jb * SB:(jb + 1) * SB],
                                         start=(et == 0), stop=(et == ET - 1))
                    # evict psum -> p_all with optional bias and per-head prescale
                    for hh in range(8):
                        head = jb * 8 + hh
                        src = ps[:, hh * HD:(hh + 1) * HD]
                        dst = p_all[:, tt, head * HD:(head + 1) * HD]
                        scale_needed = (head >= 8) or need_pre_col
                        if need_pbias:
                            tmp = small.tile([P, HD], F32, tag="pbtmp")
                            nc.vector.tensor_add(
                                out=tmp, in0=src,
                                in1=pbias[:, head * HD:(head + 1) * HD])
                            src = tmp
                        if scale_needed:
                            nc.vector.tensor_scalar_mul(
                                out=dst, in0=src,
                                scalar1=pret[:, tt, head:head + 1])
                        else:
                            nc.vector.tensor_copy(out=dst, in_=src)

        # ---------------- phase 2: causal mixing ----------------
        with sp:
            ph2 = ExitStack()
            cscale = ph2.enter_context(tc.tile_pool(name="cscale", bufs=1))
            colsc = cscale.tile([P, NPAIR // 2, T], F32, tag="colsc")
            for pr in range(NPAIR // 2):
                for hf in range(2):
                    nc.gpsimd.dma_start(
                        out=colsc[hf * HD:(hf + 1) * HD, pr, :],
                        in_=pc_d[2 * pr + hf, :].partition_broadcast(HD))
            if need_post_row:
                rowsc = cscale.tile([P, NPAIR // 2, T], F32, tag="rowsc")
                for pr in range(NPAIR // 2):
                    for hf in range(2):
                        nc.gpsimd.dma_start(
                            out=rowsc[hf * HD:(hf + 1) * HD, pr, :],
                            in_=pr_d[2 * pr + hf, :].partition_broadcast(HD))

            mxpool = sm.enter_context(tc.tile_pool(name="mxpool", bufs=1, side="right"))
            mixed = mxpool.tile([P, ET, T], F32R, tag="mixed")

            for pr in range(NPAIR):
                is_col = pr < NPAIR // 2
                carry = None
                for bs in range(NSB):
                    ps = mainps.tile([P, SB], F32, tag="mm")
                    for j in range(4):
                        kt = 4 * bs + j
                        nc.tensor.matmul(
                            ps[:, j * P:SB],
                            p_all[:, kt, pr * P:(pr + 1) * P],
                            cfull[:, 0:SB - j * P],
                            start=(j == 0), stop=(j == 3))
                    if bs < NSB - 1:
                        carry2 = small.tile([P, 1], F32, tag="carry")
                        if carry is None:
                            nc.vector.tensor_copy(out=carry2, in_=ps[:, SB - 1:SB])
                        else:
                            nc.vector.tensor_add(out=carry2, in0=ps[:, SB - 1:SB],
                                                 in1=carry)
                    dst = mixed[:, pr, bs * SB:(bs + 1) * SB]
                    if is_col:
                        tmp = small.tile([P, SB], F32, tag="mxtmp")
                        if carry is None:
                            nc.vector.tensor_mul(
                                out=dst, in0=ps[:],
                                in1=colsc[:, pr, bs * SB:(bs + 1) * SB])
                        else:
                            nc.vector.tensor_scalar_add(out=tmp, in0=ps[:],
                                                        scalar1=carry)
                            nc.vector.tensor_mul(
                                out=dst, in0=tmp,
                                in1=colsc[:, pr, bs * SB:(bs + 1) * SB])
                    else:
                        if need_post_row:
                            tmp = small.tile([P, SB], F32, tag="mxtmp")
                            if carry is None:
                                nc.vector.tensor_copy(out=tmp, in_=ps[:])
                            else:
                                nc.vector.tensor_scalar_add(out=tmp, in0=ps[:],
                                                            scalar1=carry)
                            nc.vector.tensor_mul(
                                out=dst, in0=tmp,
                                in1=rowsc[:, pr - 4, bs * SB:(bs + 1) * SB])
                        else:
                            if carry is None:
                                nc.vector.tensor_copy(out=dst, in_=ps[:])
                            else:
                                nc.vector.tensor_scalar_add(out=dst, in0=ps[:],
                                                            scalar1=carry)
                    if bs < NSB - 1:
                        carry = carry2
            ph2.close()

        # ---------------- phase 3: out-proj + residual ----------------
        x2pool = top.enter_context(tc.tile_pool(name="x2pool", bufs=1))
        x2 = x2pool.tile([P, TT, E], F32, tag="x2")
        with sm as ph:
            owpool = ph.enter_context(tc.tile_pool(name="owpool", bufs=1))
            ow_sb = owpool.tile([P, ET, E], F32R, tag="oww")
            nc.sync.dma_start(out=ow_sb, in_=ow_d[:].rearrange("(et p) f -> p et f", p=P))

            xin2 = ph.enter_context(tc.tile_pool(name="xin2", bufs=3))
            for tt in range(TT):
                x_t = xin2.tile([P, E], F32, tag="xr")
                nc.sync.dma_start(out=x_t, in_=x_d[tt * P:(tt + 1) * P, :])
                for jb in range(2):
                    ps = mainps.tile([P, SB], F32, tag="mm")
                    nmm = ET + (1 if need_opbias else 0)
                    for et in range(ET):
                        nc.tensor.matmul(ps[:], mixed[:, et, tt * P:(tt + 1) * P],
                                         ow_sb[:, et, jb * SB:(jb + 1) * SB],
                                         start=(et == 0), stop=(et == nmm - 1))
                    if need_opbias:
                        nc.tensor.matmul(ps[:], obl[:, tt * P:(tt + 1) * P],
                                         obr[:, jb * SB:(jb + 1) * SB],
                                         start=False, stop=True)
                    nc.vector.tensor_add(out=x2[:, tt, jb * SB:(jb + 1) * SB],
                                         in0=ps[:], in1=x_t[:, jb * SB:(jb + 1) * SB])

        # ---------------- phase 4: LN2 + transpose + FF ----------------
        with ExitStack() as ph:
            gpool = ph.enter_context(tc.tile_pool(name="gpool", bufs=1, side="right"))
            gt = gpool.tile([P, MT, TB * P], F32R, tag="gt")
            h2p = ph.enter_context(tc.tile_pool(name="h2p", bufs=2))
            h2tp = ph.enter_context(tc.tile_pool(name="h2tp", bufs=1))
            f1p = ph.enter_context(tc.tile_pool(name="f1p", bufs=4))
            f2p = ph.enter_context(tc.tile_pool(name="f2p", bufs=4))
            osbp = ph.enter_context(tc.tile_pool(name="osbp", bufs=3))

            for tb in range(NTB):
                h2t = h2tp.tile([P, ET, TB * P], F32R, tag="h2t")
                for tl in range(TB):
                    tt = tb * TB + tl
                    h2_t = h2p.tile([P, E], F32R, tag="h2")
                    layernorm(x2[:, tt, :], h2_t, small)
                    for ec in range(ET):
                        pst = tps.tile([P, P], F32R, tag="tp")
                        nc.tensor.transpose(pst[:], h2_t[:, ec * P:(ec + 1) * P],
                                            ident[:])
                        nc.scalar.copy(out=h2t[:, ec, tl * P:(tl + 1) * P], in_=pst[:])
                # FF1 + gelu
                for mt in range(MT):
                    f1t = f1p.tile([P, ET, P], F32R, tag="f1")
                    nc.sync.dma_start(
                        out=f1t,
                        in_=w1_d[:, mt * P:(mt + 1) * P].rearrange(
                            "(et p) m -> p et m", p=P))
                    ps = mainps.tile([P, TB * P], F32, tag="mm")
                    for et in range(ET):
                        nc.tensor.matmul(ps[:], f1t[:, et, :], h2t[:, et, :],
                                         start=(et == 0), stop=(et == ET - 1))
                    gelu_bias = 0.0 if GELU_AF == "Copy" else b1t[:, mt:mt + 1]
                    nc.scalar.activation(out=gt[:, mt, :], in_=ps[:],
                                         func=getattr(AF, GELU_AF),
                                         bias=gelu_bias, scale=1.0)
                # FF2 + residual
                for jb in range(2):
                    pss = []
                    for _ps_i in range(TB):
                        ps_ff2 = mainps.tile([P, SB], F32, tag="mm")
                        pss.append(ps_ff2)
                    for mt in range(MT):
                        f2t = f2p.tile([P, SB], F32R, tag="f2")
                        nc.sync.dma_start(
                            out=f2t,
                            in_=w2_d[mt * P:(mt + 1) * P, jb * SB:(jb + 1) * SB])
                        for tl in range(TB):
                            nc.tensor.matmul(pss[tl][:],
                                             gt[:, mt, tl * P:(tl + 1) * P], f2t[:],
                                             start=(mt == 0), stop=(mt == MT - 1))
                    for tl in range(TB):
                        tt = tb * TB + tl
                        osb = osbp.tile([P, SB], F32, tag="osb")
                        if need_b2:
                            nc.vector.tensor_add(out=osb, in0=pss[tl][:],
                                                 in1=x2[:, tt, jb * SB:(jb + 1) * SB])
                            nc.vector.tensor_add(out=osb, in0=osb,
                                                 in1=b2b[:, jb * SB:(jb + 1) * SB])
                        else:
                            nc.vector.tensor_add(out=osb, in0=pss[tl][:],
                                                 in1=x2[:, tt, jb * SB:(jb + 1) * SB])
                        nc.sync.dma_start(
                            out=out_d[tt * P:(tt + 1) * P, jb * SB:(jb + 1) * SB],
                            in_=osb)

    nc.finalize()
    return nc


def _prep(inputs):
    """Host-side folding of weights/decay. Returns (flags, per-core in_maps)."""
    f32 = np.float32
    x = np.asarray(inputs["x"], f32)
    w_proj = np.asarray(inputs["w_proj"], f32)
    b_proj = np.asarray(inputs["b_proj"], f32)
    mix_w = np.asarray(inputs["mix_w"], f32)
    mix_b = np.asarray(inputs["mix_b"], f32)
    decay = np.asarray(inputs["decay"], f32)
    out_w = np.asarray(inputs["out_w"], f32)
    out_b = np.asarray(inputs["out_b"], f32)
    ln1_g = np.asarray(inputs["ln1_g"], f32)
    ln1_b = np.asarray(inputs["ln1_b"], f32)
    ln2_g = np.asarray(inputs["ln2_g"], f32)
    ln2_b = np.asarray(inputs["ln2_b"], f32)
    ff_w1 = np.asarray(inputs["ff_w1"], f32)
    ff_b1 = np.asarray(inputs["ff_b1"], f32)
    ff_w2 = np.asarray(inputs["ff_w2"], f32)
    ff_b2 = np.asarray(inputs["ff_b2"], f32)

    wp_flat = w_proj.transpose(1, 0, 2).reshape(E, E)          # (e, h*HD)
    wp = (ln1_g[:, None] * wp_flat).astype(f32)
    p_bias = (b_proj.reshape(-1) + ln1_b @ wp_flat).astype(f32)

    d = np.clip(decay.astype(np.float64), 0.9, 1.0)            # (H,)
    jj = np.arange(T, dtype=np.float64) / DC
    a = d[:, None] ** jj[None, :]                              # (H, T)
    ainv = d[:, None] ** (-jj[None, :])
    pre = ainv.copy()
    pre[H // 2:] *= mix_w[H // 2:].astype(np.float64)
    post_col = (a[: H // 2] * mix_w[: H // 2].astype(np.float64)).astype(f32)
    post_row = a[H // 2:].astype(f32)
    pret = pre.T.astype(f32).copy()                            # (T, H)

    need_pre_col = bool((d != 1.0).any())
    need_post_row = need_pre_col
    if not need_pre_col:
        # col-head prescale is identity -> the evict for heads 0..7 copies
        pret[:, : H // 2] = 1.0
    need_pbias = bool(np.any(p_bias != 0.0))
    need_opbias = bool(np.any(mix_b != 0.0) or np.any(out_b != 0.0))
    need_b2 = bool(np.any(ff_b2 != 0.0))

    w1 = (ln2_g[:, None] * ff_w1).astype(f32)
    b1 = (ff_b1 + ln2_b @ ff_w1).astype(f32)
    b1t = b1.reshape(MT, P).T.copy()                           # (P, MT)

    cfull = (np.arange(SB)[None, :] >= np.arange(P)[:, None]).astype(f32)
    ident = np.eye(P, dtype=f32)

    common = {
        "wp": wp, "ow": out_w, "w1": w1, "w2": ff_w2,
        "cfull": cfull, "ident": ident, "pret": pret,
        "postc": post_col, "b1t": b1t,
    }
    if need_post_row:
        common["postr"] = post_row
    if need_pbias:
        common["pbias"] = p_bias.reshape(1, E)
    if need_opbias:
        obl = np.zeros((32, T), f32)
        obl[:H] = mix_b
        obl[H] = 1.0
        wbar = out_w.reshape(H, HD, E).sum(1).astype(f32)
        obr = np.zeros((32, E), f32)
        obr[:H] = wbar
        obr[H] = out_b
        common["oblhs"] = obl
        common["obrhs"] = obr
    if need_b2:
        common["b2"] = ff_b2.reshape(1, E)

    flags = (need_pre_col, need_post_row, need_pbias, need_opbias, need_b2)
    in_maps = [dict(common, x=np.ascontiguousarray(x[c])) for c in range(NCORES)]
    return flags, in_maps


def kernel(**inputs):
    from concourse.bass_utils import run_bass_kernel_spmd

    flags, in_maps = _prep(inputs)
    if flags not in _CACHE:
        _CACHE[flags] = _build(flags)
    nc = _CACHE[flags]
    res = run_bass_kernel_spmd(nc, in_maps, core_ids=list(range(NCORES)))
    out = np.stack([res.results[c]["out"] for c in range(NCORES)], axis=0)
    return out.astype(np.float32)
